# revision 49
# baseline (speedup 1.0000x reference)
"""MHSA (RoPE + causal softmax) Trainium2 Bass kernel.

Problem: x[4,2048,1024], Wq/Wk/Wv/Wo[1024,1024] fp32; 16 heads, d_k=64.

Sharding over the 8 NeuronCores: 4-way data-parallel over batch x 2-way
tensor-parallel over heads. core = 2*b + t handles batch b, heads
[t*8, t*8+8). Column-parallel Wq/Wk/Wv, row-parallel Wo; the two TP
partial outputs per batch are summed on the host (the gather step).

Device-side design (per core, all sizes hardcoded):
  - Host supplies x^T [1024,2048] (bf16) so every matmul contraction dim
    (model dim d or k-positions) lands on SBUF partitions. All matmuls
    are bf16 (1 col/cycle streaming, fp32 PSUM accumulation).
  - RoPE: interleaved even/odd pairs are pre-permuted in the Wq/Wk ROWS
    (host side) so each rotation partner lives 16 partitions away within
    a 32-partition quadrant; the rotation is then
        q' = q * cosT + stream_shuffle(q) * sinT
    with a single DVE stream-shuffle (swap 16-blocks) and sign baked
    into sinT.
  - Attention per head pair (2 heads at PE row bases 0/64):
       S^T[j] = K_j @ Q_I^T          (bf16, [128 kpos, <=512 q])
       P^T    = exp(S^T / 8)          (ACT, psum -> bf16 sbuf)
       causal: block-skip j>4I+3, narrow diagonal tiles, one [128,128]
       tri-mask multiply on the diagonal block
       O^T   += Vpk_j^T @ P^T         (bf16; V tiles packed [V0|1|V1] so
                                       each head's 128-col weight block
                                       carries 64 ones-columns: the
                                       softmax denominator rides along in
                                       the otherwise-idle output rows)
       mh^T   = O^T[vals] * approx_recip(O^T[denoms])  (recip reads PSUM)
  - Schedule: a software pipeline over head pairs p: phase p issues
    scores+exp for pair p interleaved (per 128-kpos step) with the AV
    matmuls of pair p-1 (lagged 2 steps so the previous pair's PSUM
    normalize is off the critical path). QKV-projection and output-
    projection matmuls are drip-fed one MM at a time between steps under
    an earliest-deadline budget, so the PE never idles while the ACT
    engine (exp) is the per-phase rate limiter.
  - Output projection back in [s, d] layout (lhsT = mh^T tiles) so the
    DRAM write is contiguous; host sums the two TP partials per batch.
"""
import numpy as np
import ml_dtypes

import concourse.bass as bass
from concourse import bacc
import concourse.tile as tile
import concourse.mybir as mybir
from concourse.bass_utils import run_bass_kernel_spmd

B, S, D = 4, 2048, 1024
HEADS, DK = 16, 64
THETA = 10000.0
TP, DP = 2, 4
HL = HEADS // TP            # 8 local heads per core
DL = HL * DK                # 512 local projection width
P = 128
SB = 512                    # q super-tile width
NSB = S // SB               # 4 q super-tiles (I)
NST = S // P                # 16 k-tiles (j)
NDC = D // P                # 8 contraction chunks over model dim
NPAIR = HL // 2             # 4 local head pairs
NP_ALL = NSB * NPAIR        # 16 pairs globally
AV_LAG = 3                  # AV of pair p-1 lags scores of pair p by 3 steps

f32 = mybir.dt.float32
bf16 = mybir.dt.bfloat16
SWAP16 = [(i + 16) % 32 for i in range(32)]


def _build():
    ALU = mybir.AluOpType
    FX = mybir.ActivationFunctionType
    nc = bacc.Bacc(None, target_bir_lowering=False)

    xT = nc.dram_tensor("xT", [D, S], bf16, kind="ExternalInput")
    # Q/K weights pre-blocked per 128-col projection group (so one 256KB
    # DMA makes one group runnable); V weights stay dc-major.
    wqkB = nc.dram_tensor("wqkB", [2 * NPAIR, P, NDC * P], bf16,
                          kind="ExternalInput")
    wvT = nc.dram_tensor("wvT", [D, DL], bf16, kind="ExternalInput")
    woT = nc.dram_tensor("woT", [DL, D], bf16, kind="ExternalInput")
    cosf = nc.dram_tensor("cosf", [DK, S], bf16, kind="ExternalInput")
    sinf = nc.dram_tensor("sinf", [DK, S], bf16, kind="ExternalInput")
    maskt = nc.dram_tensor("maskt", [P, P], bf16, kind="ExternalInput")
    out = nc.dram_tensor("out", [S, D], f32, kind="ExternalOutput")

    xT_t = xT.rearrange("(dc p) s -> p dc s", p=P)        # [128, 8, 2048]
    wv_t = wvT.rearrange("(dc p) f -> p dc f", p=P)       # [128, 8, 512]
    wo_t = woT.rearrange("(c p) f -> p c f", p=P)         # [128, 4, 1024]

    with tile.TileContext(nc) as tc:
        with (
            tc.tile_pool(name="wpool", bufs=1) as wpool,
            tc.tile_pool(name="kpool", bufs=1) as kpool,
            tc.tile_pool(name="xpool", bufs=2) as xpool,
            tc.tile_pool(name="qpool", bufs=3) as qpool,
            tc.tile_pool(name="tpool", bufs=2) as tpool,
            tc.tile_pool(name="ptpool", bufs=4) as ptpool,
            tc.tile_pool(name="mpool", bufs=4) as mpool,
            tc.tile_pool(name="spool", bufs=2) as spool,
            tc.tile_pool(name="opool", bufs=2) as opool,
            tc.tile_pool(name="ps_proj", bufs=2, space="PSUM") as ps_proj,
            tc.tile_pool(name="ps_s", bufs=2, space="PSUM") as ps_s,
            tc.tile_pool(name="ps_av", bufs=1, space="PSUM") as ps_av,
        ):
            # ---- head: PE warm-up + ACT exp-table preload come first so
            # neither gates on the bulky constant DMAs.
            warm = wpool.tile([P, SB], bf16)
            nc.vector.memset(warm[:], 0.0)
            warm_sink = wpool.tile([P, SB], f32)
            dummy = wpool.tile([P, 32], bf16)
            nc.scalar.activation(dummy[:], warm[:, 0:32], FX.Exp, scale=0.125)

            def emit_warm(n):
                pw = ps_s.tile([P, 2, SB], f32, tag="pss", name=f"warm{emit_warm.k}")
                emit_warm.k += 1
                for _ in range(n):
                    nc.tensor.matmul(pw[:, 0, :], warm[:, 0:P], warm[:],
                                     start=True, stop=True)
                nc.vector.tensor_copy(warm_sink[:], pw[:, 0, :])
            emit_warm.k = 0
            emit_warm(16)

            # ---- input DMAs. Never on nc.scalar (would block exp).
            xt0 = xpool.tile([P, NDC, SB], bf16, tag="xt", name="xt0")
            wqk_sb = wpool.tile([P, 2 * NPAIR, NDC, P], bf16)
            wv_sb = wpool.tile([P, NDC, DL], bf16)
            engs = (nc.sync, nc.gpsimd)
            # First two Q/K group blocks lead (each unlocks one projection
            # group), then xt0 in ONE strided DMA (groups need all 8 dc
            # chunks anyway — fewer DMA instructions beats finer grain),
            # then cos/sin (RoPE needs them early), the rest behind.
            nc.sync.dma_start(
                wqk_sb[:, 0, :, :],
                wqkB[0, :, :].rearrange("p (dc c) -> p dc c", c=P))
            nc.gpsimd.dma_start(
                wqk_sb[:, NPAIR, :, :],
                wqkB[NPAIR, :, :].rearrange("p (dc c) -> p dc c", c=P))
            for dc in range(NDC):
                engs[dc % 2].dma_start(xt0[:, dc, :], xT_t[:, dc, 0:SB])
            cos_sb = wpool.tile([P, S], bf16)
            sin_sb = wpool.tile([P, S], bf16)
            nc.gpsimd.dma_start(cos_sb[0:DK, :], cosf[:])
            nc.sync.dma_start(sin_sb[0:DK, :], sinf[:])
            nc.gpsimd.dma_start(cos_sb[DK:P, :], cosf[:])
            nc.sync.dma_start(sin_sb[DK:P, :], sinf[:])
            for k, i8 in enumerate((1, NPAIR + 1)):
                engs[k % 2].dma_start(
                    wqk_sb[:, i8, :, :], wqkB[i8, :, :].rearrange(
                        "p (dc c) -> p dc c", c=P)
                )
            for dc in range(NDC):
                engs[dc % 2].dma_start(wv_sb[:, dc, :], wv_t[:, dc, :])
            for k, i8 in enumerate((2, NPAIR + 2, 3, NPAIR + 3)):
                engs[k % 2].dma_start(
                    wqk_sb[:, i8, :, :], wqkB[i8, :, :].rearrange(
                        "p (dc c) -> p dc c", c=P)
                )
            mask_sb = wpool.tile([P, P], bf16)
            nc.gpsimd.dma_start(mask_sb[:], maskt[:])
            wo_sb = wpool.tile([P, DL // P, D], bf16)

            ktall = kpool.tile([P, NPAIR, S], bf16)
            v_sb = kpool.tile([P, NST, HL, 2 * DK], bf16)
            nc.gpsimd.memset(v_sb[:, :, :, DK : 2 * DK], 1.0)

            xts = {0: xt0}
            # prefetch xt1 behind the head DMAs (proj-1 filler pulls on it
            # during phases 0-1, before enqueue_proj(1) would emit it)
            xt1 = xpool.tile([P, NDC, SB], bf16, tag="xt", name="xt1")
            xts[1] = xt1
            for dc in range(NDC):
                engs[dc % 2].dma_start(xt1[:, dc, :], xT_t[:, dc, SB : 2 * SB])
            qts, mhs, pts, pos = {}, {}, {}, {}

            # ---------- work generators (drip-fed between pipeline steps)
            def gen_qk_group(I, fc):
                xt, qt = xts[I], qts[I]
                scol = slice(I * SB, (I + 1) * SB)
                pp = ps_proj.tile([P, SB], f32, tag="pp", name=f"pp{I}_{fc}")
                for dc in range(NDC):
                    nc.tensor.matmul(
                        pp[:],
                        wqk_sb[:, fc, dc, :],
                        xt[:, dc, :],
                        start=(dc == 0),
                        stop=(dc == NDC - 1),
                        skip_group_check=True,
                    )
                    yield
                tsh = tpool.tile([P, SB], f32, tag="tsh", name=f"tsh{I}_{fc}")
                nc.vector.stream_shuffle(tsh[:], pp[:], mask=SWAP16)
                dest = qt[:, fc, :] if fc < NPAIR else ktall[:, fc - NPAIR, scol]
                nc.vector.tensor_tensor(dest, pp[:], cos_sb[:, scol], ALU.mult)
                tsn = tpool.tile([P, SB], bf16, tag="tsn", name=f"tsn{I}_{fc}")
                nc.vector.tensor_tensor(tsn[:], tsh[:], sin_sb[:, scol], ALU.mult)
                nc.vector.tensor_tensor(dest, dest, tsn[:], ALU.add)

            def gen_v_group(I, st):
                xt = xts[I]
                pp = ps_proj.tile([P, SB], f32, tag="pp", name=f"ppv{I}_{st}")
                for dc in range(NDC):
                    nc.tensor.matmul(
                        pp[:],
                        xt[:, dc, st * P : (st + 1) * P],
                        wv_sb[:, dc, :],
                        start=(dc == 0),
                        stop=(dc == NDC - 1),
                        skip_group_check=True,
                    )
                    yield
                nc.vector.tensor_copy(
                    v_sb[:, I * 4 + st, :, 0:DK],
                    pp[:].rearrange("p (h d) -> p h d", h=HL),
                )

            def gen_outproj(I, st, tail=False):
                mh = mhs[I]
                osb = opool.tile([P, D], f32, tag="osb", name=f"osb{I}_{st}")
                row = slice((I * 4 + st) * P, (I * 4 + st + 1) * P)
                for oh in (0, 1):
                    pq = ps_proj.tile([P, SB], f32, tag="pp", name=f"pq{I}_{st}_{oh}")
                    for c2 in range(NPAIR):
                        nc.tensor.matmul(
                            pq[:],
                            mh[:, c2, st * P : (st + 1) * P],
                            wo_sb[:, c2, oh * SB : (oh + 1) * SB],
                            start=(c2 == 0),
                            stop=(c2 == NPAIR - 1),
                            skip_group_check=True,
                        )
                        yield
                    col = slice(oh * SB, (oh + 1) * SB)
                    if tail:
                        # late blocks drain during the ACT-idle tail region:
                        # copy on ScalarE and stream the DMA out per half.
                        nc.scalar.copy(osb[:, col], pq[:])
                        nc.sync.dma_start(out[row, col], osb[:, col])
                    else:
                        nc.vector.tensor_copy(osb[:, col], pq[:])
                if not tail:
                    nc.sync.dma_start(out[row, :], osb[:])

            # filler FIFO: (deadline_phase, generator), kept sorted by
            # deadline (stable), so force-drain never head-of-line blocks.
            fifo = []

            def fifo_insert(dl, g):
                idx = next(
                    (i for i, (d, _) in enumerate(fifo) if d > dl), len(fifo)
                )
                fifo.insert(idx, (dl, g))

            def enqueue_proj(I):
                if I >= NSB:
                    return
                if I not in xts:
                    xt = xpool.tile([P, NDC, SB], bf16, tag="xt", name=f"xt{I}")
                    xts[I] = xt
                    for dc in range(NDC):
                        engs[dc % 2].dma_start(
                            xt[:, dc, :], xT_t[:, dc, I * SB : (I + 1) * SB]
                        )
                if I == 1:
                    # wo behind xt1 in the queues: it is not needed until
                    # the first outproj drains (~80us in), while xt1 gates
                    # the phase-0/1 projection filler.
                    for c0 in range(DL // P):
                        engs[c0 % 2].dma_start(wo_sb[:, c0, :], wo_t[:, c0, :])
                qts[I] = qpool.tile([P, NPAIR, SB], bf16, tag="qt", name=f"qt{I}")
                base = 4 * I
                for dl, g in [
                    (base - 1, gen_qk_group(I, 0)), (base - 1, gen_qk_group(I, 4)),
                    (base + 0, gen_qk_group(I, 1)), (base + 0, gen_qk_group(I, 5)),
                    (base + 0, gen_v_group(I, 0)), (base + 0, gen_v_group(I, 1)),
                    (base + 0, gen_v_group(I, 2)), (base + 0, gen_v_group(I, 3)),
                    (base + 1, gen_qk_group(I, 2)), (base + 1, gen_qk_group(I, 6)),
                    (base + 2, gen_qk_group(I, 3)), (base + 2, gen_qk_group(I, 7)),
                ]:
                    fifo_insert(max(dl, 0), g)

            def enqueue_outproj(I):
                for st in range(SB // P):
                    fifo_insert(NP_ALL + 1, gen_outproj(I, st, tail=(I >= 2)))

            # Generators must never interleave mid-group (two open PSUM
            # accumulation groups on one pool tag would race), so a started
            # generator is held in `cur` and always finished before the next
            # one starts — fifo_insert can reorder only NOT-yet-started gens.
            cur = [None]

            def drain_one():
                while True:
                    if cur[0] is None:
                        if not fifo:
                            return False
                        cur[0] = fifo.pop(0)[1]
                    try:
                        next(cur[0])
                        return True
                    except StopIteration:
                        cur[0] = None

            def drain_deadline(ph):
                while fifo and fifo[0][0] <= ph:
                    drain_one()

            # steps remaining at the start of each phase (for budgeting)
            def phase_steps(p):
                n_sc = 4 * (p // 4) + 4 if p < NP_ALL else 0
                n_av = 4 * ((p - 1) // 4) + 4 if p >= 1 else 0
                return max(n_sc, (n_av + AV_LAG) if n_av else 0)

            steps_after = [0] * (NP_ALL + 3)
            for p in range(NP_ALL, -1, -1):
                steps_after[p] = steps_after[p + 1] + phase_steps(p)

            # earliest-deadline-first filler pacing; each gen ~8 MMs
            def budget(p, step_in_phase):
                if not fifo:
                    return 0.0
                best, cum = 0.0, 0
                for d, _g in fifo:
                    cum += 8
                    du = (steps_after[p] - steps_after[min(d, NP_ALL + 1)]) \
                        - step_in_phase
                    if du <= 0:
                        return 8.0
                    best = max(best, cum / du)
                return best

            def emit_scores_step(p, j):
                I, c = divmod(p, 4)
                m = j - 4 * I
                off = m * P if m > 0 else 0
                N = SB - off
                pss = ps_s.tile([P, 2, SB], f32, tag="pss", name=f"pss{p}_{j}")
                for half in (0, 1):
                    pr = 64 * half
                    nc.tensor.matmul(
                        pss[:, half, :N],
                        ktall[pr : pr + 64, c, j * P : (j + 1) * P],
                        qts[I][pr : pr + 64, c, off:SB],
                        start=True,
                        stop=True,
                    )
                pt = ptpool.tile(
                    [P, 2, SB], bf16, tag="pt", name=f"pt{p}_{j}", bufs=18
                )
                nc.scalar.activation(pt[:, :, :N], pss[:, :, :N], FX.Exp, scale=0.125)
                if m >= 0:
                    nc.vector.tensor_tensor(
                        pt[:, :, 0:P], pt[:, :, 0:P],
                        mask_sb[:, None, :].to_broadcast((P, 2, P)),
                        ALU.mult,
                    )
                pts[(p, j)] = (pt, off, N)

            def emit_av_step(p, j, njt):
                I, c = divmod(p, 4)
                if j == 0:
                    pos[p] = (
                        ps_av.tile([P, SB], f32, tag="po0", name=f"po{p}_0", bufs=1),
                        ps_av.tile([P, SB], f32, tag="po1", name=f"po{p}_1", bufs=1),
                    )
                po0, po1 = pos[p]
                pt, off, N = pts.pop((p, j))
                for half, po in ((0, po0), (1, po1)):
                    nc.tensor.matmul(
                        po[:, off:SB],
                        v_sb[:, j, 2 * c + half, :],
                        pt[:, half, :N],
                        start=(j == 0),
                        stop=(j == njt - 1),
                        skip_group_check=True,
                    )

            def emit_normalize(p):
                I, c = divmod(p, 4)
                if c == 0:
                    mhs[I] = mpool.tile([P, NPAIR, SB], bf16, tag="mh", name=f"mh{I}")
                mhI = mhs[I]
                po0, po1 = pos.pop(p)
                for half, po in ((0, po0), (1, po1)):
                    # reciprocal_approx_fast gives wrong results on a PSUM
                    # source (hardware-verified) — stage through SBUF. The
                    # staging copy runs on ScalarE to keep the DVE queue
                    # (RoPE + outproj copies) off the po-recycle path.
                    lsb = spool.tile([64, SB], f32, tag="lsb", name=f"lsb{p}_{half}")
                    nc.scalar.copy(lsb[:], po[DK:P, :])
                    rec = spool.tile([64, SB], f32, tag="rec", name=f"rec{p}_{half}")
                    nc.vector.reciprocal_approx_fast(rec[:], lsb[:])
                    nc.vector.tensor_tensor(
                        mhI[64 * half : 64 * half + 64, c, :], po[0:DK, :], rec[:],
                        ALU.mult,
                    )

            # ---------- pre-phase: block-0 projection runs dense, with
            # dependency-free warm matmuls interleaved as elastic filler —
            # the drain is DMA-arrival-gated at the head, and the warm MMs
            # occupy exactly the stall windows (and hold the HAM clock up).
            enqueue_proj(0)
            for _ in range(8):
                emit_warm(2)
                for _ in range(12):
                    if not drain_one():
                        break
            drain_deadline(3)

            # ---------- attention pipeline over pairs.
            debt = 0.0
            for p in range(NP_ALL + 1):
                has_sc = p < NP_ALL
                n_sc = 4 * (p // 4) + 4 if has_sc else 0
                n_av = 4 * ((p - 1) // 4) + 4 if p >= 1 else 0
                drain_deadline(p)             # clear overdue producer work
                if has_sc and p % 4 == 0:
                    enqueue_proj(p // 4 + 1)
                steps = phase_steps(p)
                for s in range(steps):
                    if has_sc and s < n_sc:
                        emit_scores_step(p, s)
                    if p >= 1 and AV_LAG <= s < n_av + AV_LAG:
                        emit_av_step(p - 1, s - AV_LAG, n_av)
                    debt += budget(p, s)
                    while debt >= 1.0 and fifo:
                        if not drain_one():
                            break
                        debt -= 1.0
                    if not fifo:
                        debt = 0.0
                if p >= 1:
                    emit_normalize(p - 1)
                    if (p - 1) % 4 == 3:
                        enqueue_outproj((p - 1) // 4)
            # tail: remaining outproj(3) chunks
            while drain_one():
                pass
    nc.finalize()
    return nc


_NC = None


def _get_nc():
    global _NC
    if _NC is None:
        _NC = _build()
    return _NC


def _host_prep(Wq, Wk, Wv, Wo):
    t = np.arange(DK // 2)
    qd, rd = t // 16, t % 16
    perm = np.empty(DK, np.int64)
    perm[qd * 32 + rd] = 2 * t
    perm[qd * 32 + 16 + rd] = 2 * t + 1

    Wq_p = Wq.reshape(HEADS, DK, D)[:, perm, :].reshape(HEADS * DK, D)
    Wk_p = Wk.reshape(HEADS, DK, D)[:, perm, :].reshape(HEADS * DK, D)

    pos = np.arange(S, dtype=np.float64)
    inv = 1.0 / THETA ** (np.arange(0, DK, 2).astype(np.float64) / DK)  # [32]
    ang = inv[:, None] * pos[None, :]                                   # [32, S]
    cos32 = np.cos(ang).astype(np.float32)
    sin32 = np.sin(ang).astype(np.float32)
    cosf = np.empty((DK, S), np.float32)
    sinf = np.empty((DK, S), np.float32)
    rows_lo = qd * 32 + rd
    rows_hi = qd * 32 + 16 + rd
    cosf[rows_lo] = cos32[t]
    cosf[rows_hi] = cos32[t]
    sinf[rows_lo] = -sin32[t]
    sinf[rows_hi] = sin32[t]

    mask01 = (
        np.arange(P)[:, None] <= np.arange(P)[None, :]
    ).astype(ml_dtypes.bfloat16)

    per_tp = []
    for tp in range(TP):
        sl = slice(tp * DL, (tp + 1) * DL)
        wqkT = np.concatenate([Wq_p[sl], Wk_p[sl]], axis=0).T  # [D, 2*DL]
        # per-128-col group blocks, each [P, NDC*P] with partition-major
        # contiguous layout so one DMA fills one group's weights
        nb = 2 * DL // P
        wqkB = np.ascontiguousarray(
            wqkT.reshape(D, nb, P).transpose(1, 0, 2)          # [8, D, P]
            .reshape(nb, NDC, P, P).transpose(0, 2, 1, 3)      # [8, P, NDC, P]
            .reshape(nb, P, NDC * P)
        ).astype(ml_dtypes.bfloat16)
        wvT = np.ascontiguousarray(Wv[sl].T).astype(ml_dtypes.bfloat16)
        woT = np.ascontiguousarray(Wo[:, sl].T).astype(ml_dtypes.bfloat16)
        per_tp.append((wqkB, wvT, woT))
    return per_tp, cosf.astype(ml_dtypes.bfloat16), sinf.astype(ml_dtypes.bfloat16), mask01


def kernel(x, Wq, Wk, Wv, Wo):
    x = np.asarray(x, np.float32)
    Wq = np.asarray(Wq, np.float32)
    Wk = np.asarray(Wk, np.float32)
    Wv = np.asarray(Wv, np.float32)
    Wo = np.asarray(Wo, np.float32)

    per_tp, cosf, sinf, mask01 = _host_prep(Wq, Wk, Wv, Wo)
    xTs = [np.ascontiguousarray(x[b].T).astype(ml_dtypes.bfloat16) for b in range(B)]

    in_maps = []
    for core in range(DP * TP):
        b, tp = core // TP, core % TP
        wqkB, wvT, woT = per_tp[tp]
        in_maps.append(
            {
                "xT": xTs[b],
                "wqkB": wqkB,
                "wvT": wvT,
                "woT": woT,
                "cosf": cosf,
                "sinf": sinf,
                "maskt": mask01,
            }
        )

    nc = _get_nc()
    res = run_bass_kernel_spmd(nc, in_maps, core_ids=list(range(DP * TP)))
    out = np.empty((B, S, D), np.float32)
    for b in range(B):
        out[b] = res.results[b * TP]["out"] + res.results[b * TP + 1]["out"]
    return out
